# revision 76
# baseline (speedup 1.0000x reference)
"""Trainium2 Bass kernel for nn_Attention_12970801234663 (V3).

Module: GroupNorm(32) -> 1x1 conv qkv -> 8-head attention over hw=1024 with the
original torch module's raw (b, heads, hw, head_dim) -> (b, c, h, w) reshape ->
1x1 out conv -> residual.

Sharding: pure data-parallel over batch (b=8) across 8 NeuronCores; weights are
broadcast. Each core computes one image end-to-end; no collectives.

Design (driven by the cost model: matmul time ~ out-free-size x rate, bf16 rate
1.0 at any size, f32r 1.0 only when free >= 256; ldweights free; the Act engine
is the only exp engine and paces the kernel at ~66us; each DMA instruction
costs a fixed ~625ns on the serialized HWDGE):
  - GroupNorm: row sums on DVE, sums of squares via Act Square+accum_out,
    group reduction via tiny PE matmuls on indicator matrices.
  - qkv pass A (q,k as [channel, hw]): PE matmuls; bias added during the DVE
    PSUM->SBUF eviction.  Pass B (v^T as [hw, channel]): two hw-chunks share
    one PSUM tile; K=1 ones-row bias preload; evicted to bf16 vt tiles
    [j, (jc, h, 33)] = [v^T | ones-col].
  - sim^T[j, i] = k^T q per head: K=32 matmuls on tile_position row groups.
  - exp on Act straight out of PSUM, bf16 out (softmax max-subtraction skipped:
    sim is O(1), and softmax is shift-invariant).
  - attn@v restructured: stationary = exp-tile column chunk [j, 128i] (bf16),
    moving = vt [j, 33] (bf16), accumulated over j-chunks into per-(head,
    ic-half) PSUM zt tiles laid out [i, (ic, d|den)].  This directly produces
    the transposed layout the module's reshape needs.
  - softmax divide fused with the zt eviction (reciprocal of the ones-column
    sums, broadcast multiply, bf16 out), per head as soon as its last j-chunk
    lands, so the scramble DMA for head h overlaps head h+1's exps.
  - scramble reshape via DRAM bounce: one scatter DMA per head (4-dim DRAM
    access pattern), one contiguous readback per 4-head group.
  - out projection with bf16 moving operand; bias + residual fused in one DVE
    scalar_tensor_tensor per output chunk.
  - The whole (pr, jc) attention stream is software-pipelined (attn@v lags one
    step behind sim/exp) and junk warm-up matmuls keep the PE p-state ramped
    where it matters.
"""
import os
import sys

for p in ("/opt/trn_rl_repo",):
    if p not in sys.path and os.path.isdir(p):
        sys.path.insert(0, p)

import copy as _copy
import ml_dtypes
import numpy as np

import concourse.bass as bass
import concourse.tile as tile
from concourse import mybir
from concourse.bass_utils import run_bass_kernel_spmd
from concourse.bass_interp import get_hw_module

F32 = mybir.dt.float32
RDT = mybir.dt.float32r   # full-rate fp32 matmul operands
BF16 = mybir.dt.bfloat16
ALU = mybir.AluOpType
AFT = mybir.ActivationFunctionType

N_CORES = 8
B, C, H, W = 8, 256, 32, 32
HW = H * W                # 1024
N_HEADS = 8
HEAD_DIM = 32
GROUPS = 32
EPS = 1e-5
SCALE = HEAD_DIM ** -0.5
GROUP_SZ = (C // GROUPS) * HW  # 8192 elements per group

# fp32 consts tensor column layout
COL_NWB = 0      # 4 cols: norm_w tile0, norm_w tile1, norm_b tile0, norm_b tile1
COL_GIND = 4     # 16 cols: [128,16] group indicator
COL_GINDT = 20   # 128 cols: rows 0:16 hold the [16,128] broadcast indicator
COL_BQK = 148    # 4 cols: qkv bias for q,k per 128-row chunk (q part scaled)
COL_BO = 152     # 2 cols: out bias per 128-row chunk
CW = 154
# f32r consts tensor row-0 layout (matmul operands)
RCOL_ONES = 0    # 512 cols of ones in row 0
RCOL_BV = 512    # 256 cols: qkv bias for v
RCOL_BQK = 768   # 512 cols: qkv bias for q,k (q part pre-scaled)
RCOL_BO = 1280   # 256 cols: out bias
RCOL_Z = 1536    # 512 zero cols (PSUM bank-zeroing matmul source)
CWR = 2048


def _split_excess_waits(m):
    """Walrus in this toolchain accepts only one sem-wait per instruction;
    move excess waits onto preceding wait-only drains on the same engine."""
    n_split = 0
    for function in m.functions:
        new_blocks = []
        for block in function.blocks:
            new_insts = []
            for ins in block.instructions:
                si = ins.sync_info
                if si is None:
                    new_insts.append(ins)
                    continue
                waits = list(si.on_wait)
                if len(waits) > 1:
                    k = 0
                    while len(waits) > 1:
                        chunk, waits = waits[:1], waits[1:]
                        d = mybir.InstDrain(
                            name=f"{ins.name}-wsplit{k}",
                            ins=[], outs=[], bass_is_fusable=False,
                        )
                        d.engine = ins.engine
                        d.sync_info = mybir.SyncInfo(on_wait=chunk, on_update=[])
                        new_insts.append(d)
                        k += 1
                        n_split += 1
                    ins.sync_info = mybir.SyncInfo(
                        on_wait=waits, on_update=list(si.on_update))
                new_insts.append(ins)
            new_blocks.append(_copy.replace(block, instructions=new_insts))
        function.blocks.clear()
        function.blocks.extend(new_blocks)
    return n_split


def build_program(fix_for_hw=True):
    nc = bass.Bass("TRN2", target_bir_lowering=False, debug=False,
                   enable_asserts=False, num_devices=N_CORES)

    x_in = nc.dram_tensor("x_in", [C, HW], RDT, kind="ExternalInput")
    wqk_in = nc.dram_tensor("wqk", [C, 512], RDT, kind="ExternalInput")
    wv_in = nc.dram_tensor("wv", [C, 256], RDT, kind="ExternalInput")
    wo_in = nc.dram_tensor("wo", [C, 256], BF16, kind="ExternalInput")
    consts_in = nc.dram_tensor("consts", [128, CW], F32, kind="ExternalInput")
    constsr_in = nc.dram_tensor("constsr", [1, CWR], RDT,
                                kind="ExternalInput")
    eyer_in = nc.dram_tensor("eyer", [128, 128], RDT, kind="ExternalInput")
    y_out = nc.dram_tensor("y_out", [C, HW], F32, kind="ExternalOutput")

    N_WARM = int(os.environ.get("K_WARM", "0"))
    BIGB = int(os.environ.get("K_BIGB", "3"))

    ctx_lp = nc.allow_low_precision("bf16/f32r matmul operands by design")
    ctx_lp.__enter__()
    with tile.TileContext(nc) as tc:
        with (
            tc.tile_pool(name="persist", bufs=1) as persist,
            tc.tile_pool(name="expp", bufs=int(os.environ.get("K_EB", "4"))) as expp,
            tc.tile_pool(name="scratch", bufs=2) as scratch,
            tc.tile_pool(name="psump", bufs=1, space="PSUM") as psump,
            tc.tile_pool(name="dramp", bufs=1, space="DRAM") as dramp,
        ):
            x_sb = persist.tile([128, 2, HW], RDT)
            xn_sb = persist.tile([128, 2, HW], RDT)
            for t in range(2):
                nc.sync.dma_start(x_sb[:, t, :], x_in[128 * t:128 * (t + 1), :])
            eyer = persist.tile([128, 128], RDT)
            nc.sync.dma_start(eyer[:], eyer_in[:])
            zt_ab = [psump.tile([128, 8, 64], F32, name=f"ztab{i}")
                     for i in range(2)]
            consts = persist.tile([128, CW], F32)
            nc.sync.dma_start(consts[:], consts_in[:])
            constsr = persist.tile([1, CWR], RDT)
            nc.sync.dma_start(constsr[:], constsr_in[:])


            def emit_junk(n, tgt=None):
                # keep the PE dispatch stream busy to hold the p-state ramp;
                # writes to scratch PSUM (or unused cols of a live zt bank)
                for _ in range(n):
                    if tgt is None:
                        jt = psump.tile([128, 64], F32, tag="big",
                                        bufs=BIGB, name="junk")
                        ap = jt[:]
                    else:
                        ap = tgt
                    nc.tensor.matmul(ap, consts[0:1, 0:128],
                                     consts[0:1, 0:64], start=True,
                                     stop=True, skip_group_check=True)

            wqk = persist.tile([128, 2, 512], RDT)
            wv = persist.tile([128, 2, 256], RDT)
            wo = persist.tile([128, 2, 256], BF16)
            for t in range(2):
                nc.sync.dma_start(wqk[:, t, :], wqk_in[128 * t:128 * (t + 1), :])
                nc.sync.dma_start(wv[:, t, :], wv_in[128 * t:128 * (t + 1), :])
                nc.sync.dma_start(wo[:, t, :], wo_in[128 * t:128 * (t + 1), :])

            # vt: [j-part, jc, head, 33] bf16 = [v^T | ones column]
            vt = persist.tile([128, 8, N_HEADS, 33], BF16)
            nc.gpsimd.memset(vt[:, :, :, 32:33], 1.0)

            emit_junk(N_WARM)

            # ---------------- GroupNorm ----------------
            gind = consts[:, COL_GIND:COL_GIND + 16]
            gindT = consts[0:16, COL_GINDT:COL_GINDT + 128]
            ab_t = {}
            for t in range(2):
                s_tile = scratch.tile([128, 2], F32, tag="gn_s")
                nc.vector.reduce_sum(s_tile[:, 0:1], x_sb[:, t, :],
                                     axis=mybir.AxisListType.X)
                xsq = scratch.tile([128, HW], F32, tag="gn_sq")
                nc.scalar.activation(xsq[:], x_sb[:, t, :], AFT.Square,
                                     accum_out=s_tile[:, 1:2])
                gsum = zt_ab[0][0:16, t, 0:2]  # bank A
                nc.tensor.matmul(gsum, gind, s_tile[:])
                st = scratch.tile([16, 2], F32, tag="gn_st")
                nc.vector.tensor_scalar(st[:], gsum, 1.0 / GROUP_SZ, None,
                                        ALU.mult)
                mu_rs = scratch.tile([16, 2], F32, tag="gn_mr")
                nc.vector.tensor_copy(mu_rs[:, 0:1], st[:, 0:1])
                var_t = scratch.tile([16, 1], F32, tag="gn_var")
                nc.vector.tensor_tensor(var_t[:], st[:, 0:1], st[:, 0:1],
                                        ALU.mult)
                nc.vector.tensor_tensor(var_t[:], st[:, 1:2], var_t[:],
                                        ALU.subtract)
                nc.vector.tensor_scalar_add(var_t[:], var_t[:], EPS)
                ln_t = scratch.tile([16, 1], F32, tag="gn_ln")
                nc.scalar.activation(ln_t[:], var_t[:], AFT.Ln)
                nc.scalar.activation(mu_rs[:, 1:2], ln_t[:], AFT.Exp,
                                     scale=-0.5)
                bc = zt_ab[1][:, 2 + t, 0:2]  # bank B
                nc.tensor.matmul(bc, gindT, mu_rs[:])
                ab = scratch.tile([128, 2], F32, tag=f"gn_ab{t}")
                # A = rsqrt * w ; B = b - mu * A
                nc.vector.tensor_tensor(ab[:, 0:1], bc[:, 1:2],
                                        consts[:, COL_NWB + t:COL_NWB + t + 1],
                                        ALU.mult)
                tmp_b = scratch.tile([128, 1], F32, tag="gn_tmp")
                nc.vector.tensor_tensor(tmp_b[:], bc[:, 0:1], ab[:, 0:1],
                                        ALU.mult)
                nc.vector.tensor_tensor(
                    ab[:, 1:2],
                    consts[:, COL_NWB + 2 + t:COL_NWB + 3 + t], tmp_b[:],
                    ALU.subtract)
                ab_t[t] = ab

            for ch in range(2):
                for t in range(2):
                    nc.vector.tensor_scalar(
                        xn_sb[:, t, 512 * ch:512 * (ch + 1)],
                        x_sb[:, t, 512 * ch:512 * (ch + 1)],
                        ab_t[t][:, 0:1], ab_t[t][:, 1:2],
                        ALU.mult, ALU.add)

            qk_sb = persist.tile([128, 4, HW], RDT)

            # pipeline sentinels: 4 cheap matmuls that wait on xn so the PE
            # WAIT_QUEUE fills here and the real qkv work is visited (costed)
            # after the warm-up ramp instead of at t~0.5us
            for _ in range(4):
                js = psump.tile([128, 384], F32, tag="big", bufs=BIGB,
                                name="sent")
                nc.tensor.matmul(js[0:32, 0:64], xn_sb[0:1, 1, 0:32],
                                 constsr[0:1, 0:64], start=True, stop=True,
                                 skip_group_check=True)

            def emit_qk_half(m, n, ps_box):
                # half of q,k chunk m (columns 512n..): bias preloaded on the
                # PE via a K=1 ones-row matmul, PSUM evicted on the (otherwise
                # idle) Act engine so the DVE stays clear for the pipeline
                if n == 0:
                    ps_box[m] = psump.tile([128, HW], F32, tag="big",
                                           bufs=BIGB, name=f"qkps{m}")
                ps = ps_box[m]
                for kc in range(2):
                    nc.tensor.matmul(
                        ps[:, 512 * n:512 * (n + 1)],
                        wqk[:, kc, 128 * m:128 * (m + 1)],
                        xn_sb[:, kc, 512 * n:512 * (n + 1)],
                        start=(kc == 0), stop=(kc == 1))
                nc.vector.tensor_scalar(
                    qk_sb[:, m, 512 * n:512 * (n + 1)],
                    ps[:, 512 * n:512 * (n + 1)],
                    consts[:, COL_BQK + m:COL_BQK + m + 1], None, ALU.add)

            qk_box = {}

            def emit_qk(m):
                for n in range(2):
                    emit_qk_half(m, n, qk_box)

            vp_box = {}

            def emit_v_half(cp, half, tag="big"):
                # v^T chunk 2cp+half as [hw 128, channel 256] -> bf16 vt
                cch = 2 * cp + half
                if tag == "zt":
                    sl = zt_ab[1][:, 4 * half:4 * (half + 1), :].rearrange(
                        "p a b -> p (a b)")
                else:
                    if half == 0:
                        vp_box[cp] = psump.tile([128, 512], F32, tag=tag,
                                                bufs=BIGB, name=f"vps{cp}")
                    sl = vp_box[cp][:, 256 * half:256 * (half + 1)]
                if half == 0 and tag == "zt":
                    nc.tensor.matmul(
                        zt_ab[1][:].rearrange("p a b -> p (a b)"),
                        constsr[0:1, RCOL_ONES:RCOL_ONES + 128],
                        constsr[0:1, RCOL_Z:RCOL_Z + 512],
                        start=True, stop=False, skip_group_check=True)
                elif half == 0:
                    nc.tensor.matmul(
                        vp_box[cp][:],
                        constsr[0:1, RCOL_ONES:RCOL_ONES + 128],
                        constsr[0:1, RCOL_Z:RCOL_Z + 512],
                        start=True, stop=False, skip_group_check=True)
                nc.tensor.matmul(sl,
                                 constsr[0:1, RCOL_ONES:RCOL_ONES + 128],
                                 constsr[0:1, RCOL_BV:RCOL_BV + 256],
                                 start=False, stop=False,
                                 skip_group_check=True)
                for kc in range(2):
                    nc.tensor.matmul(
                        sl,
                        xn_sb[:, kc, 128 * cch:128 * (cch + 1)],
                        wv[:, kc, :], start=False,
                        stop=(kc == 1 and half == 1),
                        skip_group_check=True)
                nc.vector.tensor_copy(
                    vt[:, cch, :, 0:32],
                    sl.rearrange("p (h d) -> p h d", d=32))

            # head q,k: matmuls for both chunks per n-half; k-chunk (m2) n0
            # evicted first so sim jc0-3 unblocks after three evictions
            qk_box[0] = psump.tile([128, HW], F32, tag="big", bufs=BIGB,
                                   name="qkps0")
            qk_box[2] = psump.tile([128, HW], F32, tag="big", bufs=BIGB,
                                   name="qkps2")

            def head_qk_half(n):
                for m in (0, 2):
                    for kc in range(2):
                        nc.tensor.matmul(
                            qk_box[m][:, 512 * n:512 * (n + 1)],
                            wqk[:, kc, 128 * m:128 * (m + 1)],
                            xn_sb[:, kc, 512 * n:512 * (n + 1)],
                            start=(kc == 0), stop=(kc == 1))
                for m in (2, 0):
                    nc.vector.tensor_scalar(
                        qk_sb[:, m, 512 * n:512 * (n + 1)],
                        qk_box[m][:, 512 * n:512 * (n + 1)],
                        consts[:, COL_BQK + m:COL_BQK + m + 1], None,
                        ALU.add)

            head_qk_half(0)
            head_qk_half(1)
            emit_v_half(0, 0, tag="zt")
            emit_v_half(0, 1, tag="zt")

            # ---------------- attention ----------------
            fillers = {
                0: lambda: (emit_qk_half(1, 0, qk_box),
                            emit_qk_half(3, 0, qk_box)),
                1: lambda: emit_qk_half(1, 1, qk_box),
                2: lambda: emit_v_half(1, 0),
                3: lambda: emit_v_half(1, 1),
                4: lambda: emit_v_half(2, 0),
                5: lambda: emit_v_half(2, 1),
                6: lambda: emit_v_half(3, 0),
                7: lambda: emit_v_half(3, 1),
                8: lambda: emit_qk_half(3, 1, qk_box),
            }

            # z_sb per group: [i-part, chi(=ic), gg(=h%4), d] bf16
            z_sb = [persist.tile([128, 8, 4, 32], BF16, name=f"z_sb{g}")
                    for g in range(2)]
            a_sb = [persist.tile([128, HW], BF16, name=f"a_sb{g}")
                    for g in range(2)]
            # per-head DRAM bounce, stored chi-major so the readback into
            # a_sb rows [32gg:32gg+32] is fully contiguous
            z_dram = [dramp.tile([8, 4, 32, 32], BF16, name=f"z_dram{h}")
                      for h in range(N_HEADS)]

            zt_tiles = {}

            def emit_sim(heads, jc, nsl=2):
                sims = {}
                w = HW // nsl
                for h in heads:
                    mq, po = h // 4, 32 * (h % 4)
                    q_ap = qk_sb[po:po + 32, mq, :]
                    k_ap = qk_sb[po:po + 32, 2 + mq, :]
                    sim = psump.tile([128, HW], F32, tag="big", bufs=BIGB,
                                     name=f"sim_{h}_{jc}")
                    for n in range(nsl):
                        nc.tensor.matmul(
                            sim[:, w * n:w * (n + 1)],
                            k_ap[:, 128 * jc:128 * (jc + 1)],
                            q_ap[:, w * n:w * (n + 1)],
                            start=True, stop=True,
                            tile_position=(po, 0))
                    sims[h] = sim
                return sims

            def emit_exp(heads, jc, sims, nsl=1):
                es = {}
                w = HW // nsl
                for h in heads:
                    e = expp.tile([128, HW], BF16, tag="e", name=f"e_{h}_{jc}")
                    for n in range(nsl):
                        nc.scalar.activation(
                            e[:, w * n:w * (n + 1)],
                            sims[h][:, w * n:w * (n + 1)], AFT.Exp)
                    es[h] = e
                return es

            def emit_attnv(heads, jc, es):
                if jc == 0:
                    for h in heads:
                        zt_tiles[h] = zt_ab[h % 2]
                        # zero + start the whole accumulator bank in one K=1
                        # matmul; all per-ic series then accumulate into it
                        nc.tensor.matmul(
                            zt_tiles[h][:].rearrange("p a b -> p (a b)"),
                            constsr[0:1, RCOL_ONES:RCOL_ONES + 128],
                            constsr[0:1, RCOL_Z:RCOL_Z + 512],
                            start=True, stop=False, skip_group_check=True)
                for h in heads:
                    e = es[h]
                    for ic in range(8):
                        zt = zt_tiles[h]
                        nc.tensor.matmul(
                            zt[:, ic, 0:33],
                            e[:, 128 * ic:128 * (ic + 1)],
                            vt[:, jc, h, :],
                            # start_tensor_calc zeroes the whole PSUM bank;
                            # the bank was started by the zeroing matmul above
                            start=False, stop=(jc == 7 and ic == 7),
                            skip_group_check=True)

            def emit_head_finish(h, tail=False):
                # head h accumulation complete: divide by denominators into
                # z (bf16), then scatter its 32 channels to the DRAM bounce
                g, gg = h // 4, h % 4
                zt = zt_tiles.pop(h)
                rec = scratch.tile([128, 8], F32, tag="rec")
                nc.vector.reciprocal(rec[:], zt[:, :, 32])
                nc.vector.tensor_tensor(
                    z_sb[g][:, :, gg, :], zt[:, :, 0:32],
                    rec[:].unsqueeze(2).broadcast_to((128, 8, 32)),
                    ALU.mult)
                # scatter: z_dram[h][chi, bb, il, d] = z_sb[g][32bb+il, chi, gg, d]
                zd = z_dram[h].rearrange("chi bb il d -> bb il chi d")
                scat_q = nc.scalar if (tail and gg % 2 == 0) else nc.sync
                rb_q = (nc.scalar if gg % 2 == 0 else nc.sync) if tail \
                    else nc.gpsimd
                scat_q.dma_start(zd, z_sb[g][:, :, gg, :])
                # contiguous readback: a_sb rows 32gg+4chi+bb, cols 32il+d
                rb_q.dma_start(
                    a_sb[g][32 * gg:32 * gg + 32],
                    z_dram[h].rearrange("chi bb il d -> (chi bb) (il d)"))

            steps = [((2 * pr, 2 * pr + 1), jc)
                     for pr in (0, 2, 3) for jc in range(8)]
            steps += [((h,), jc) for jc in range(8) for h in (2, 3)]
            last_k = len(steps) - 1
            pend = []
            for k, (heads, jc) in enumerate(steps):
                if k == 0:
                    sims = emit_sim(heads, jc)
                    es = emit_exp(heads, jc, sims, nsl=2)
                else:
                    sims = emit_sim(heads, jc)
                    es = emit_exp(heads, jc, sims,
                                  nsl=2 if k >= last_k - 1 else 1)
                pend.append((heads, jc, es))
                if k >= 1:
                    fl = fillers.pop(k - 1, None)
                    if fl:
                        fl()
                    pheads, pjc, pes = pend.pop(0)
                    emit_attnv(pheads, pjc, pes)
                    if pjc == 7:
                        for h in pheads:
                            emit_head_finish(h, tail=(h // 2 == 1))
            pheads, pjc, pes = pend.pop(0)
            emit_attnv(pheads, pjc, pes)
            for h in pheads:
                emit_head_finish(h, tail=True)

            # ---------------- out projection + residual ----------------
            # kc=1 accumulated first (its readbacks landed long ago); the
            # kc=0 matmul waiting on the tail readbacks comes last in each
            # series.  Residual+bias fused per 512-column half, stores
            # alternate between the SP and Act DMA queues.
            ps_m = {}
            for m in range(2):
                ps_m[m] = psump.tile([128, HW], F32, tag="big", bufs=BIGB,
                                     name=f"ops{m}")
            # early parts: out bias (K=1 ones row), the residual x (identity
            # matmul), kc=1 (group 1, landed long ago) and kc=0 rows 0:64
            # (heads 0,1); only the K=64 rows 64:128 part waits on the tail
            # readbacks of heads 2,3.  PSUM is evicted on the by-then idle
            # Act engine, so the DVE plays no part in the tail.
            for m in range(2):
                for n in range(2):
                    sl = ps_m[m][:, 512 * n:512 * (n + 1)]
                    nc.tensor.matmul(
                        sl,
                        constsr[0:1, RCOL_BO + 128 * m:
                                RCOL_BO + 128 * (m + 1)],
                        constsr[0:1, RCOL_ONES:RCOL_ONES + 512],
                        start=True, stop=False)
                    nc.tensor.matmul(
                        sl, eyer[:], x_sb[:, m, 512 * n:512 * (n + 1)],
                        start=False, stop=False)
                    nc.tensor.matmul(
                        sl, wo[:, 1, 128 * m:128 * (m + 1)],
                        a_sb[1][:, 512 * n:512 * (n + 1)],
                        start=False, stop=False)
                    nc.tensor.matmul(
                        sl, wo[0:64, 0, 128 * m:128 * (m + 1)],
                        a_sb[0][0:64, 512 * n:512 * (n + 1)],
                        start=False, stop=False)
            for m in range(2):
                for n in range(2):
                    sl = ps_m[m][:, 512 * n:512 * (n + 1)]
                    nc.tensor.matmul(
                        sl, wo[64:128, 0, 128 * m:128 * (m + 1)],
                        a_sb[0][64:128, 512 * n:512 * (n + 1)],
                        start=False, stop=True)
                    y_sb = scratch.tile([128, 512], F32, tag="y_sb", bufs=4,
                                        name=f"y_{m}_{n}")
                    if m == 0:
                        nc.scalar.activation(y_sb[:], sl, AFT.Copy)
                    else:
                        nc.vector.tensor_copy(y_sb[:], sl)
                    q = nc.sync if (2 * m + n) % 2 == 0 else nc.scalar
                    q.dma_start(
                        y_out[128 * m:128 * (m + 1), 512 * n:512 * (n + 1)],
                        y_sb[:])

    ctx_lp.__exit__(None, None, None)
    nc.finalize()
    if fix_for_hw:
        nc.m = get_hw_module(nc.m)
        _split_excess_waits(nc.m)
    return nc


def host_prep(x, norm_w, norm_b, qkv_w, qkv_b, out_w, out_b):
    """Build per-core input maps from full inputs."""
    x = np.asarray(x, np.float32)
    qkv_w = np.asarray(qkv_w, np.float32)
    qkv_b = np.asarray(qkv_b, np.float32)
    out_w = np.asarray(out_w, np.float32)
    out_b = np.asarray(out_b, np.float32)
    norm_w = np.asarray(norm_w, np.float32)
    norm_b = np.asarray(norm_b, np.float32)

    wT = np.ascontiguousarray(qkv_w.T)          # [256, 768]
    wqk = wT[:, 0:512].copy()
    wqk[:, 0:256] *= SCALE
    bqk = qkv_b[0:512].copy()
    bqk[0:256] *= SCALE
    wv = np.ascontiguousarray(wT[:, 512:768])
    bv = qkv_b[512:768]
    wo = np.ascontiguousarray(out_w.T).astype(ml_dtypes.bfloat16)  # [256, 256]

    consts = np.zeros((128, CW), np.float32)
    consts[:, COL_NWB + 0] = norm_w[0:128]
    consts[:, COL_NWB + 1] = norm_w[128:256]
    consts[:, COL_NWB + 2] = norm_b[0:128]
    consts[:, COL_NWB + 3] = norm_b[128:256]
    p = np.arange(128)
    consts[p, COL_GIND + p // 8] = 1.0
    consts[p // 8, COL_GINDT + p] = 1.0  # rows 0:16
    for m in range(4):
        consts[:, COL_BQK + m] = bqk[128 * m:128 * (m + 1)]
    for m in range(2):
        consts[:, COL_BO + m] = out_b[128 * m:128 * (m + 1)]

    constsr = np.zeros((1, CWR), np.float32)
    constsr[0, RCOL_ONES:RCOL_ONES + 512] = 1.0
    constsr[0, RCOL_BV:RCOL_BV + 256] = bv
    constsr[0, RCOL_BQK:RCOL_BQK + 512] = bqk
    constsr[0, RCOL_BO:RCOL_BO + 256] = out_b

    shared = {"wqk": wqk, "wv": wv, "wo": wo, "consts": consts,
              "constsr": constsr, "eyer": np.eye(128, dtype=np.float32)}
    in_maps = []
    for b in range(N_CORES):
        m = dict(shared)
        m["x_in"] = np.ascontiguousarray(x[b].reshape(C, HW))
        in_maps.append(m)
    return in_maps


_PROGRAM = None


def _get_program():
    global _PROGRAM
    if _PROGRAM is None:
        _PROGRAM = build_program()
    return _PROGRAM


def kernel(x, norm_w, norm_b, qkv_w, qkv_b, out_w, out_b, _trace=False):
    nc = _get_program()
    in_maps = host_prep(x, norm_w, norm_b, qkv_w, qkv_b, out_w, out_b)
    res = run_bass_kernel_spmd(nc, in_maps, list(range(N_CORES)), trace=_trace)
    out = np.stack([res.results[b]["y_out"].reshape(C, H, W)
                    for b in range(N_CORES)])
    if _trace:
        kernel.last_result = res
    return out.astype(np.float32)


# revision 77
# speedup vs baseline: 1.0053x; 1.0053x over previous
"""Trainium2 Bass kernel for nn_Attention_12970801234663 (V3).

Module: GroupNorm(32) -> 1x1 conv qkv -> 8-head attention over hw=1024 with the
original torch module's raw (b, heads, hw, head_dim) -> (b, c, h, w) reshape ->
1x1 out conv -> residual.

Sharding: pure data-parallel over batch (b=8) across 8 NeuronCores; weights are
broadcast. Each core computes one image end-to-end; no collectives.

Design (driven by the cost model: matmul time ~ out-free-size x rate, bf16 rate
1.0 at any size, f32r 1.0 only when free >= 256; ldweights free; the Act engine
is the only exp engine and paces the kernel at ~66us; each DMA instruction
costs a fixed ~625ns on the serialized HWDGE):
  - GroupNorm: row sums on DVE, sums of squares via Act Square+accum_out,
    group reduction via tiny PE matmuls on indicator matrices.
  - qkv pass A (q,k as [channel, hw]): PE matmuls; bias added during the DVE
    PSUM->SBUF eviction.  Pass B (v^T as [hw, channel]): two hw-chunks share
    one PSUM tile; K=1 ones-row bias preload; evicted to bf16 vt tiles
    [j, (jc, h, 33)] = [v^T | ones-col].
  - sim^T[j, i] = k^T q per head: K=32 matmuls on tile_position row groups.
  - exp on Act straight out of PSUM, bf16 out (softmax max-subtraction skipped:
    sim is O(1), and softmax is shift-invariant).
  - attn@v restructured: stationary = exp-tile column chunk [j, 128i] (bf16),
    moving = vt [j, 33] (bf16), accumulated over j-chunks into per-(head,
    ic-half) PSUM zt tiles laid out [i, (ic, d|den)].  This directly produces
    the transposed layout the module's reshape needs.
  - softmax divide fused with the zt eviction (reciprocal of the ones-column
    sums, broadcast multiply, bf16 out), per head as soon as its last j-chunk
    lands, so the scramble DMA for head h overlaps head h+1's exps.
  - scramble reshape via DRAM bounce: one scatter DMA per head (4-dim DRAM
    access pattern), one contiguous readback per 4-head group.
  - out projection with bf16 moving operand; bias + residual fused in one DVE
    scalar_tensor_tensor per output chunk.
  - The whole (pr, jc) attention stream is software-pipelined (attn@v lags one
    step behind sim/exp) and junk warm-up matmuls keep the PE p-state ramped
    where it matters.
"""
import os
import sys

for p in ("/opt/trn_rl_repo",):
    if p not in sys.path and os.path.isdir(p):
        sys.path.insert(0, p)

import copy as _copy
import ml_dtypes
import numpy as np

import concourse.bass as bass
import concourse.tile as tile
from concourse import mybir
from concourse.bass_utils import run_bass_kernel_spmd
from concourse.bass_interp import get_hw_module

F32 = mybir.dt.float32
RDT = mybir.dt.float32r   # full-rate fp32 matmul operands
BF16 = mybir.dt.bfloat16
ALU = mybir.AluOpType
AFT = mybir.ActivationFunctionType

N_CORES = 8
B, C, H, W = 8, 256, 32, 32
HW = H * W                # 1024
N_HEADS = 8
HEAD_DIM = 32
GROUPS = 32
EPS = 1e-5
SCALE = HEAD_DIM ** -0.5
GROUP_SZ = (C // GROUPS) * HW  # 8192 elements per group

# fp32 consts tensor column layout
COL_NWB = 0      # 4 cols: norm_w tile0, norm_w tile1, norm_b tile0, norm_b tile1
COL_GIND = 4     # 16 cols: [128,16] group indicator
COL_GINDT = 20   # 128 cols: rows 0:16 hold the [16,128] broadcast indicator
COL_BQK = 148    # 4 cols: qkv bias for q,k per 128-row chunk (q part scaled)
COL_BO = 152     # 2 cols: out bias per 128-row chunk
CW = 154
# f32r consts tensor row-0 layout (matmul operands)
RCOL_ONES = 0    # 512 cols of ones in row 0
RCOL_BV = 512    # 256 cols: qkv bias for v
RCOL_BQK = 768   # 512 cols: qkv bias for q,k (q part pre-scaled)
RCOL_BO = 1280   # 256 cols: out bias
RCOL_Z = 1536    # 512 zero cols (PSUM bank-zeroing matmul source)
CWR = 2048


def _split_excess_waits(m):
    """Walrus in this toolchain accepts only one sem-wait per instruction;
    move excess waits onto preceding wait-only drains on the same engine."""
    n_split = 0
    for function in m.functions:
        new_blocks = []
        for block in function.blocks:
            new_insts = []
            for ins in block.instructions:
                si = ins.sync_info
                if si is None:
                    new_insts.append(ins)
                    continue
                waits = list(si.on_wait)
                if len(waits) > 1:
                    k = 0
                    while len(waits) > 1:
                        chunk, waits = waits[:1], waits[1:]
                        d = mybir.InstDrain(
                            name=f"{ins.name}-wsplit{k}",
                            ins=[], outs=[], bass_is_fusable=False,
                        )
                        d.engine = ins.engine
                        d.sync_info = mybir.SyncInfo(on_wait=chunk, on_update=[])
                        new_insts.append(d)
                        k += 1
                        n_split += 1
                    ins.sync_info = mybir.SyncInfo(
                        on_wait=waits, on_update=list(si.on_update))
                new_insts.append(ins)
            new_blocks.append(_copy.replace(block, instructions=new_insts))
        function.blocks.clear()
        function.blocks.extend(new_blocks)
    return n_split


def build_program(fix_for_hw=True):
    nc = bass.Bass("TRN2", target_bir_lowering=False, debug=False,
                   enable_asserts=False, num_devices=N_CORES)

    x_in = nc.dram_tensor("x_in", [C, HW], RDT, kind="ExternalInput")
    wqk_in = nc.dram_tensor("wqk", [C, 512], RDT, kind="ExternalInput")
    wv_in = nc.dram_tensor("wv", [C, 256], RDT, kind="ExternalInput")
    wo_in = nc.dram_tensor("wo", [C, 256], BF16, kind="ExternalInput")
    consts_in = nc.dram_tensor("consts", [128, CW], F32, kind="ExternalInput")
    constsr_in = nc.dram_tensor("constsr", [1, CWR], RDT,
                                kind="ExternalInput")
    eyer_in = nc.dram_tensor("eyer", [128, 128], RDT, kind="ExternalInput")
    y_out = nc.dram_tensor("y_out", [C, HW], F32, kind="ExternalOutput")

    N_WARM = int(os.environ.get("K_WARM", "0"))
    BIGB = int(os.environ.get("K_BIGB", "3"))

    ctx_lp = nc.allow_low_precision("bf16/f32r matmul operands by design")
    ctx_lp.__enter__()
    with tile.TileContext(nc) as tc:
        with (
            tc.tile_pool(name="persist", bufs=1) as persist,
            tc.tile_pool(name="expp", bufs=int(os.environ.get("K_EB", "4"))) as expp,
            tc.tile_pool(name="scratch", bufs=2) as scratch,
            tc.tile_pool(name="psump", bufs=1, space="PSUM") as psump,
            tc.tile_pool(name="dramp", bufs=1, space="DRAM") as dramp,
        ):
            x_sb = persist.tile([128, 2, HW], RDT)
            xn_sb = persist.tile([128, 2, HW], RDT)
            for t in range(2):
                nc.sync.dma_start(x_sb[:, t, :], x_in[128 * t:128 * (t + 1), :])
            zt_ab = [psump.tile([128, 8, 64], F32, name=f"ztab{i}")
                     for i in range(2)]
            consts = persist.tile([128, CW], F32)
            nc.sync.dma_start(consts[:], consts_in[:])
            constsr = persist.tile([1, CWR], RDT)
            nc.sync.dma_start(constsr[:], constsr_in[:])
            eyer = persist.tile([128, 128], RDT)


            def emit_junk(n, tgt=None):
                # keep the PE dispatch stream busy to hold the p-state ramp;
                # writes to scratch PSUM (or unused cols of a live zt bank)
                for _ in range(n):
                    if tgt is None:
                        jt = psump.tile([128, 64], F32, tag="big",
                                        bufs=BIGB, name="junk")
                        ap = jt[:]
                    else:
                        ap = tgt
                    nc.tensor.matmul(ap, consts[0:1, 0:128],
                                     consts[0:1, 0:64], start=True,
                                     stop=True, skip_group_check=True)

            wqk = persist.tile([128, 2, 512], RDT)
            wv = persist.tile([128, 2, 256], RDT)
            wo = persist.tile([128, 2, 256], BF16)
            for t in range(2):
                nc.sync.dma_start(wqk[:, t, :], wqk_in[128 * t:128 * (t + 1), :])
                nc.sync.dma_start(wv[:, t, :], wv_in[128 * t:128 * (t + 1), :])
                nc.sync.dma_start(wo[:, t, :], wo_in[128 * t:128 * (t + 1), :])
            nc.sync.dma_start(eyer[:], eyer_in[:])

            # vt: [j-part, jc, head, 33] bf16 = [v^T | ones column]
            vt = persist.tile([128, 8, N_HEADS, 33], BF16)
            nc.gpsimd.memset(vt[:, :, :, 32:33], 1.0)

            emit_junk(N_WARM)

            # ---------------- GroupNorm ----------------
            gind = consts[:, COL_GIND:COL_GIND + 16]
            gindT = consts[0:16, COL_GINDT:COL_GINDT + 128]
            ab_t = {}
            for t in range(2):
                s_tile = scratch.tile([128, 2], F32, tag="gn_s")
                nc.vector.reduce_sum(s_tile[:, 0:1], x_sb[:, t, :],
                                     axis=mybir.AxisListType.X)
                xsq = scratch.tile([128, HW], F32, tag="gn_sq")
                nc.scalar.activation(xsq[:], x_sb[:, t, :], AFT.Square,
                                     accum_out=s_tile[:, 1:2])
                gsum = zt_ab[0][0:16, t, 0:2]  # bank A
                nc.tensor.matmul(gsum, gind, s_tile[:])
                st = scratch.tile([16, 2], F32, tag="gn_st")
                nc.vector.tensor_scalar(st[:], gsum, 1.0 / GROUP_SZ, None,
                                        ALU.mult)
                mu_rs = scratch.tile([16, 2], F32, tag="gn_mr")
                nc.vector.tensor_copy(mu_rs[:, 0:1], st[:, 0:1])
                var_t = scratch.tile([16, 1], F32, tag="gn_var")
                nc.vector.tensor_tensor(var_t[:], st[:, 0:1], st[:, 0:1],
                                        ALU.mult)
                nc.vector.tensor_tensor(var_t[:], st[:, 1:2], var_t[:],
                                        ALU.subtract)
                nc.vector.tensor_scalar_add(var_t[:], var_t[:], EPS)
                ln_t = scratch.tile([16, 1], F32, tag="gn_ln")
                nc.scalar.activation(ln_t[:], var_t[:], AFT.Ln)
                nc.scalar.activation(mu_rs[:, 1:2], ln_t[:], AFT.Exp,
                                     scale=-0.5)
                bc = zt_ab[1][:, 2 + t, 0:2]  # bank B
                nc.tensor.matmul(bc, gindT, mu_rs[:])
                ab = scratch.tile([128, 2], F32, tag=f"gn_ab{t}")
                # A = rsqrt * w ; B = b - mu * A
                nc.vector.tensor_tensor(ab[:, 0:1], bc[:, 1:2],
                                        consts[:, COL_NWB + t:COL_NWB + t + 1],
                                        ALU.mult)
                tmp_b = scratch.tile([128, 1], F32, tag="gn_tmp")
                nc.vector.tensor_tensor(tmp_b[:], bc[:, 0:1], ab[:, 0:1],
                                        ALU.mult)
                nc.vector.tensor_tensor(
                    ab[:, 1:2],
                    consts[:, COL_NWB + 2 + t:COL_NWB + 3 + t], tmp_b[:],
                    ALU.subtract)
                ab_t[t] = ab

            for ch in range(2):
                for t in range(2):
                    nc.vector.tensor_scalar(
                        xn_sb[:, t, 512 * ch:512 * (ch + 1)],
                        x_sb[:, t, 512 * ch:512 * (ch + 1)],
                        ab_t[t][:, 0:1], ab_t[t][:, 1:2],
                        ALU.mult, ALU.add)

            qk_sb = persist.tile([128, 4, HW], RDT)

            # pipeline sentinels: 4 cheap matmuls that wait on xn so the PE
            # WAIT_QUEUE fills here and the real qkv work is visited (costed)
            # after the warm-up ramp instead of at t~0.5us
            for _ in range(4):
                js = psump.tile([128, 384], F32, tag="big", bufs=BIGB,
                                name="sent")
                nc.tensor.matmul(js[0:32, 0:64], xn_sb[0:1, 1, 0:32],
                                 constsr[0:1, 0:64], start=True, stop=True,
                                 skip_group_check=True)

            def emit_qk_half(m, n, ps_box):
                # half of q,k chunk m (columns 512n..): bias preloaded on the
                # PE via a K=1 ones-row matmul, PSUM evicted on the (otherwise
                # idle) Act engine so the DVE stays clear for the pipeline
                if n == 0:
                    ps_box[m] = psump.tile([128, HW], F32, tag="big",
                                           bufs=BIGB, name=f"qkps{m}")
                ps = ps_box[m]
                for kc in range(2):
                    nc.tensor.matmul(
                        ps[:, 512 * n:512 * (n + 1)],
                        wqk[:, kc, 128 * m:128 * (m + 1)],
                        xn_sb[:, kc, 512 * n:512 * (n + 1)],
                        start=(kc == 0), stop=(kc == 1))
                nc.vector.tensor_scalar(
                    qk_sb[:, m, 512 * n:512 * (n + 1)],
                    ps[:, 512 * n:512 * (n + 1)],
                    consts[:, COL_BQK + m:COL_BQK + m + 1], None, ALU.add)

            qk_box = {}

            def emit_qk(m):
                for n in range(2):
                    emit_qk_half(m, n, qk_box)

            vp_box = {}

            def emit_v_half(cp, half, tag="big"):
                # v^T chunk 2cp+half as [hw 128, channel 256] -> bf16 vt
                cch = 2 * cp + half
                if tag == "zt":
                    sl = zt_ab[1][:, 4 * half:4 * (half + 1), :].rearrange(
                        "p a b -> p (a b)")
                else:
                    if half == 0:
                        vp_box[cp] = psump.tile([128, 512], F32, tag=tag,
                                                bufs=BIGB, name=f"vps{cp}")
                    sl = vp_box[cp][:, 256 * half:256 * (half + 1)]
                if half == 0 and tag == "zt":
                    nc.tensor.matmul(
                        zt_ab[1][:].rearrange("p a b -> p (a b)"),
                        constsr[0:1, RCOL_ONES:RCOL_ONES + 128],
                        constsr[0:1, RCOL_Z:RCOL_Z + 512],
                        start=True, stop=False, skip_group_check=True)
                elif half == 0:
                    nc.tensor.matmul(
                        vp_box[cp][:],
                        constsr[0:1, RCOL_ONES:RCOL_ONES + 128],
                        constsr[0:1, RCOL_Z:RCOL_Z + 512],
                        start=True, stop=False, skip_group_check=True)
                nc.tensor.matmul(sl,
                                 constsr[0:1, RCOL_ONES:RCOL_ONES + 128],
                                 constsr[0:1, RCOL_BV:RCOL_BV + 256],
                                 start=False, stop=False,
                                 skip_group_check=True)
                for kc in range(2):
                    nc.tensor.matmul(
                        sl,
                        xn_sb[:, kc, 128 * cch:128 * (cch + 1)],
                        wv[:, kc, :], start=False,
                        stop=(kc == 1 and half == 1),
                        skip_group_check=True)
                nc.vector.tensor_copy(
                    vt[:, cch, :, 0:32],
                    sl.rearrange("p (h d) -> p h d", d=32))

            # head q,k: matmuls for both chunks per n-half; k-chunk (m2) n0
            # evicted first so sim jc0-3 unblocks after three evictions
            qk_box[0] = psump.tile([128, HW], F32, tag="big", bufs=BIGB,
                                   name="qkps0")
            qk_box[2] = psump.tile([128, HW], F32, tag="big", bufs=BIGB,
                                   name="qkps2")

            def head_qk_half(n):
                for m in (0, 2):
                    for kc in range(2):
                        nc.tensor.matmul(
                            qk_box[m][:, 512 * n:512 * (n + 1)],
                            wqk[:, kc, 128 * m:128 * (m + 1)],
                            xn_sb[:, kc, 512 * n:512 * (n + 1)],
                            start=(kc == 0), stop=(kc == 1))
                for m in (2, 0):
                    nc.vector.tensor_scalar(
                        qk_sb[:, m, 512 * n:512 * (n + 1)],
                        qk_box[m][:, 512 * n:512 * (n + 1)],
                        consts[:, COL_BQK + m:COL_BQK + m + 1], None,
                        ALU.add)

            head_qk_half(0)
            head_qk_half(1)
            emit_v_half(0, 0, tag="zt")
            emit_v_half(0, 1, tag="zt")

            # ---------------- attention ----------------
            fillers = {
                0: lambda: (emit_qk_half(1, 0, qk_box),
                            emit_qk_half(3, 0, qk_box)),
                1: lambda: emit_qk_half(1, 1, qk_box),
                2: lambda: emit_v_half(1, 0),
                3: lambda: emit_v_half(1, 1),
                4: lambda: emit_v_half(2, 0),
                5: lambda: emit_v_half(2, 1),
                6: lambda: emit_v_half(3, 0),
                7: lambda: emit_v_half(3, 1),
                8: lambda: emit_qk_half(3, 1, qk_box),
            }

            # z_sb per group: [i-part, chi(=ic), gg(=h%4), d] bf16
            z_sb = [persist.tile([128, 8, 4, 32], BF16, name=f"z_sb{g}")
                    for g in range(2)]
            a_sb = [persist.tile([128, HW], BF16, name=f"a_sb{g}")
                    for g in range(2)]
            # per-head DRAM bounce, stored chi-major so the readback into
            # a_sb rows [32gg:32gg+32] is fully contiguous
            z_dram = [dramp.tile([8, 4, 32, 32], BF16, name=f"z_dram{h}")
                      for h in range(N_HEADS)]

            zt_tiles = {}

            def emit_sim(heads, jc, nsl=2):
                sims = {}
                w = HW // nsl
                for h in heads:
                    mq, po = h // 4, 32 * (h % 4)
                    q_ap = qk_sb[po:po + 32, mq, :]
                    k_ap = qk_sb[po:po + 32, 2 + mq, :]
                    sim = psump.tile([128, HW], F32, tag="big", bufs=BIGB,
                                     name=f"sim_{h}_{jc}")
                    for n in range(nsl):
                        nc.tensor.matmul(
                            sim[:, w * n:w * (n + 1)],
                            k_ap[:, 128 * jc:128 * (jc + 1)],
                            q_ap[:, w * n:w * (n + 1)],
                            start=True, stop=True,
                            tile_position=(po, 0))
                    sims[h] = sim
                return sims

            def emit_exp(heads, jc, sims, nsl=1):
                es = {}
                w = HW // nsl
                for h in heads:
                    e = expp.tile([128, HW], BF16, tag="e", name=f"e_{h}_{jc}")
                    for n in range(nsl):
                        nc.scalar.activation(
                            e[:, w * n:w * (n + 1)],
                            sims[h][:, w * n:w * (n + 1)], AFT.Exp)
                    es[h] = e
                return es

            def emit_attnv(heads, jc, es):
                if jc == 0:
                    for h in heads:
                        zt_tiles[h] = zt_ab[h % 2]
                        # zero + start the whole accumulator bank in one K=1
                        # matmul; all per-ic series then accumulate into it
                        nc.tensor.matmul(
                            zt_tiles[h][:].rearrange("p a b -> p (a b)"),
                            constsr[0:1, RCOL_ONES:RCOL_ONES + 128],
                            constsr[0:1, RCOL_Z:RCOL_Z + 512],
                            start=True, stop=False, skip_group_check=True)
                for h in heads:
                    e = es[h]
                    for ic in range(8):
                        zt = zt_tiles[h]
                        nc.tensor.matmul(
                            zt[:, ic, 0:33],
                            e[:, 128 * ic:128 * (ic + 1)],
                            vt[:, jc, h, :],
                            # start_tensor_calc zeroes the whole PSUM bank;
                            # the bank was started by the zeroing matmul above
                            start=False, stop=(jc == 7 and ic == 7),
                            skip_group_check=True)

            def emit_head_finish(h, tail=False):
                # head h accumulation complete: divide by denominators into
                # z (bf16), then scatter its 32 channels to the DRAM bounce
                g, gg = h // 4, h % 4
                zt = zt_tiles.pop(h)
                rec = scratch.tile([128, 8], F32, tag="rec")
                nc.vector.reciprocal(rec[:], zt[:, :, 32])
                nc.vector.tensor_tensor(
                    z_sb[g][:, :, gg, :], zt[:, :, 0:32],
                    rec[:].unsqueeze(2).broadcast_to((128, 8, 32)),
                    ALU.mult)
                # scatter: z_dram[h][chi, bb, il, d] = z_sb[g][32bb+il, chi, gg, d]
                zd = z_dram[h].rearrange("chi bb il d -> bb il chi d")
                scat_q = nc.scalar if (tail and gg % 2 == 0) else nc.sync
                rb_q = (nc.scalar if gg % 2 == 0 else nc.sync) if tail \
                    else nc.gpsimd
                scat_q.dma_start(zd, z_sb[g][:, :, gg, :])
                # contiguous readback: a_sb rows 32gg+4chi+bb, cols 32il+d
                rb_q.dma_start(
                    a_sb[g][32 * gg:32 * gg + 32],
                    z_dram[h].rearrange("chi bb il d -> (chi bb) (il d)"))

            steps = [((2 * pr, 2 * pr + 1), jc)
                     for pr in (0, 2, 3) for jc in range(8)]
            steps += [((h,), jc) for jc in range(8) for h in (2, 3)]
            last_k = len(steps) - 1
            pend = []
            for k, (heads, jc) in enumerate(steps):
                if k == 0:
                    sims = emit_sim(heads, jc)
                    es = emit_exp(heads, jc, sims, nsl=2)
                else:
                    sims = emit_sim(heads, jc)
                    es = emit_exp(heads, jc, sims,
                                  nsl=2 if k >= last_k - 1 else 1)
                pend.append((heads, jc, es))
                if k >= 1:
                    fl = fillers.pop(k - 1, None)
                    if fl:
                        fl()
                    pheads, pjc, pes = pend.pop(0)
                    emit_attnv(pheads, pjc, pes)
                    if pjc == 7:
                        for h in pheads:
                            emit_head_finish(h, tail=(h // 2 == 1))
            pheads, pjc, pes = pend.pop(0)
            emit_attnv(pheads, pjc, pes)
            for h in pheads:
                emit_head_finish(h, tail=True)

            # ---------------- out projection + residual ----------------
            # kc=1 accumulated first (its readbacks landed long ago); the
            # kc=0 matmul waiting on the tail readbacks comes last in each
            # series.  Residual+bias fused per 512-column half, stores
            # alternate between the SP and Act DMA queues.
            ps_m = {}
            for m in range(2):
                ps_m[m] = psump.tile([128, HW], F32, tag="big", bufs=BIGB,
                                     name=f"ops{m}")
            # early parts: out bias (K=1 ones row), the residual x (identity
            # matmul), kc=1 (group 1, landed long ago) and kc=0 rows 0:64
            # (heads 0,1); only the K=64 rows 64:128 part waits on the tail
            # readbacks of heads 2,3.  PSUM is evicted on the by-then idle
            # Act engine, so the DVE plays no part in the tail.
            for m in range(2):
                for n in range(2):
                    sl = ps_m[m][:, 512 * n:512 * (n + 1)]
                    nc.tensor.matmul(
                        sl,
                        constsr[0:1, RCOL_BO + 128 * m:
                                RCOL_BO + 128 * (m + 1)],
                        constsr[0:1, RCOL_ONES:RCOL_ONES + 512],
                        start=True, stop=False)
                    nc.tensor.matmul(
                        sl, eyer[:], x_sb[:, m, 512 * n:512 * (n + 1)],
                        start=False, stop=False)
                    nc.tensor.matmul(
                        sl, wo[:, 1, 128 * m:128 * (m + 1)],
                        a_sb[1][:, 512 * n:512 * (n + 1)],
                        start=False, stop=False)
                    nc.tensor.matmul(
                        sl, wo[0:64, 0, 128 * m:128 * (m + 1)],
                        a_sb[0][0:64, 512 * n:512 * (n + 1)],
                        start=False, stop=False)
            for m in range(2):
                for n in range(2):
                    sl = ps_m[m][:, 512 * n:512 * (n + 1)]
                    nc.tensor.matmul(
                        sl, wo[64:128, 0, 128 * m:128 * (m + 1)],
                        a_sb[0][64:128, 512 * n:512 * (n + 1)],
                        start=False, stop=True)
                    y_sb = scratch.tile([128, 512], F32, tag="y_sb", bufs=4,
                                        name=f"y_{m}_{n}")
                    if m == 0:
                        nc.scalar.activation(y_sb[:], sl, AFT.Copy)
                    else:
                        nc.vector.tensor_copy(y_sb[:], sl)
                    q = nc.sync if (2 * m + n) % 2 == 0 else nc.scalar
                    q.dma_start(
                        y_out[128 * m:128 * (m + 1), 512 * n:512 * (n + 1)],
                        y_sb[:])

    ctx_lp.__exit__(None, None, None)
    nc.finalize()
    if fix_for_hw:
        nc.m = get_hw_module(nc.m)
        _split_excess_waits(nc.m)
    return nc


def host_prep(x, norm_w, norm_b, qkv_w, qkv_b, out_w, out_b):
    """Build per-core input maps from full inputs."""
    x = np.asarray(x, np.float32)
    qkv_w = np.asarray(qkv_w, np.float32)
    qkv_b = np.asarray(qkv_b, np.float32)
    out_w = np.asarray(out_w, np.float32)
    out_b = np.asarray(out_b, np.float32)
    norm_w = np.asarray(norm_w, np.float32)
    norm_b = np.asarray(norm_b, np.float32)

    wT = np.ascontiguousarray(qkv_w.T)          # [256, 768]
    wqk = wT[:, 0:512].copy()
    wqk[:, 0:256] *= SCALE
    bqk = qkv_b[0:512].copy()
    bqk[0:256] *= SCALE
    wv = np.ascontiguousarray(wT[:, 512:768])
    bv = qkv_b[512:768]
    wo = np.ascontiguousarray(out_w.T).astype(ml_dtypes.bfloat16)  # [256, 256]

    consts = np.zeros((128, CW), np.float32)
    consts[:, COL_NWB + 0] = norm_w[0:128]
    consts[:, COL_NWB + 1] = norm_w[128:256]
    consts[:, COL_NWB + 2] = norm_b[0:128]
    consts[:, COL_NWB + 3] = norm_b[128:256]
    p = np.arange(128)
    consts[p, COL_GIND + p // 8] = 1.0
    consts[p // 8, COL_GINDT + p] = 1.0  # rows 0:16
    for m in range(4):
        consts[:, COL_BQK + m] = bqk[128 * m:128 * (m + 1)]
    for m in range(2):
        consts[:, COL_BO + m] = out_b[128 * m:128 * (m + 1)]

    constsr = np.zeros((1, CWR), np.float32)
    constsr[0, RCOL_ONES:RCOL_ONES + 512] = 1.0
    constsr[0, RCOL_BV:RCOL_BV + 256] = bv
    constsr[0, RCOL_BQK:RCOL_BQK + 512] = bqk
    constsr[0, RCOL_BO:RCOL_BO + 256] = out_b

    shared = {"wqk": wqk, "wv": wv, "wo": wo, "consts": consts,
              "constsr": constsr, "eyer": np.eye(128, dtype=np.float32)}
    in_maps = []
    for b in range(N_CORES):
        m = dict(shared)
        m["x_in"] = np.ascontiguousarray(x[b].reshape(C, HW))
        in_maps.append(m)
    return in_maps


_PROGRAM = None


def _get_program():
    global _PROGRAM
    if _PROGRAM is None:
        _PROGRAM = build_program()
    return _PROGRAM


def kernel(x, norm_w, norm_b, qkv_w, qkv_b, out_w, out_b, _trace=False):
    nc = _get_program()
    in_maps = host_prep(x, norm_w, norm_b, qkv_w, qkv_b, out_w, out_b)
    res = run_bass_kernel_spmd(nc, in_maps, list(range(N_CORES)), trace=_trace)
    out = np.stack([res.results[b]["y_out"].reshape(C, H, W)
                    for b in range(N_CORES)])
    if _trace:
        kernel.last_result = res
    return out.astype(np.float32)
